# revision 23
# baseline (speedup 1.0000x reference)
"""Multi-head attention (RoPE + causal softmax) Trainium2 Bass kernel.

Problem: nn_MultiHeadAttention (B=16, S=512, D=1024, H=16, Hd=64).
Sharding: data-parallel over batch — 2 batches per core on 8 NeuronCores.

Feature-major device layout ([feature, token] tiles) so the contraction
always sits on SBUF partitions.  Per-core pipeline: q/k projections with
RoPE, v projection (ones-column augmented so attn@v also yields softmax
denominators), per-head-pair causal attention with row-group-packed score
matmuls, Wo projection.

RoPE uses the tan identity: because the RoPE tables repeat with period 32,
R @ (q * tan) * cos == rotate_half(q) * sin, so the rotation matmul
accumulates directly into the projection PSUM (start=False continuation)
and RoPE costs just 2 DVE ops per group.  The causal diagonal mask is a
single [128, 2, 128] bf16 DVE multiply covering both heads of a pair.

v7 scheduling (from trace analysis):
- Each dma_start trigger costs ~0.5us of serial Sync-engine time, and one
  ring's packets already spread across all 16 DMA engines — so inputs are
  staged host-side as contiguous slabs and loaded with ~22 big triggers
  ordered critical-first (wq/wk m=0 slices, batch-0 half of x, cos/tan,
  then the rest).  First real matmul ~9us instead of ~24us.
- PE warmup matmuls on the RT tile (first loaded) bridge the preamble to
  data arrival so HAM stays at K=8/8; the ACT exp table pre-loads in the
  same window.
- Softmax denominator reciprocal runs on the [128,8] transposed layout
  (DVE reciprocal costs ~6.5 cyc/elem per lane), one transpose dance per
  pair, denominator copies split Scalar/Vector.
- wo output copies on Scalar; output stored bf16 (halves out-DMA bytes).
- Global schedule gives every attention pair ~5.5us of co-emitted dense
  matmul work so the softmax chain latency hides.
"""

import numpy as np
import ml_dtypes

BF16 = ml_dtypes.bfloat16

B, S, D = 16, 512, 1024
H, HD = 16, 64
NCORES = 8
BPC = B // NCORES
T = BPC * S

_CACHE = {}


def _rope_tables():
    inv_freq = 1.0 / (10000.0 ** (np.arange(0, HD, 2, dtype=np.float64) / HD))
    t = np.arange(S, dtype=np.float64)
    freqs = np.outer(t, inv_freq)
    emb = np.concatenate([freqs, freqs], -1)
    return np.cos(emb), np.sin(emb)


def _host_consts():
    cos, sin = _rope_tables()
    tan = sin / cos
    cosT = np.tile(cos.T, (2, 1))
    tanT = np.tile(tan.T, (2, 1))
    costan = np.ascontiguousarray(
        np.concatenate([cosT, tanT], axis=1)).astype(BF16)  # [128, 2S]
    R64 = np.zeros((64, 64), np.float32)
    R64[np.arange(32), np.arange(32) + 32] = -1.0
    R64[np.arange(32) + 32, np.arange(32)] = 1.0
    R128 = np.zeros((128, 128), np.float32)
    R128[:64, :64] = R64
    R128[64:, 64:] = R64
    RT = R128.T.astype(BF16)
    mask01 = (np.arange(128)[None, :] >= np.arange(128)[:, None]).astype(BF16)
    mask2 = np.concatenate([mask01, mask01], axis=1)     # [128, 256]
    rtmask = np.ascontiguousarray(
        np.concatenate([RT, mask2], axis=1))              # [128, 384]
    return costan, rtmask


def _block_weight(w):
    """WqT [D, D] -> [8, 128, 1024] m-major contiguous slabs:
    out[m, p, k*128 + c] = WqT[k*128+p, m*128+c]."""
    wt = np.ascontiguousarray(w.T).astype(BF16)
    b = wt.reshape(8, 128, 8, 128)                        # [k, p, m, c]
    return np.ascontiguousarray(
        b.transpose(2, 1, 0, 3).reshape(8, 128, 1024))    # [m][p, k, c]


def _kmajor(w):
    """WvT [D, D] -> [128, 8, 1024]: out[p, k, c] = WvT[k*128+p, c]."""
    wt = np.ascontiguousarray(w.T).astype(BF16)
    return np.ascontiguousarray(
        wt.reshape(8, 128, 1024).transpose(1, 0, 2))


def _build_bass(dump_debug=False):
    import concourse.bacc as bacc
    import concourse.tile as tile
    import concourse.mybir as mybir

    dt = mybir.dt
    f32, bf16 = dt.float32, dt.bfloat16
    Exp = mybir.ActivationFunctionType.Exp

    nc = bacc.Bacc("TRN2", target_bir_lowering=False, debug=False, enable_asserts=False)

    xT_d = nc.dram_tensor("xTw", [2, 128, 8, S], bf16, kind="ExternalInput").ap()
    wq_d = nc.dram_tensor("WqB", [8, 128, 1024], bf16, kind="ExternalInput").ap()
    wk_d = nc.dram_tensor("WkB", [8, 128, 1024], bf16, kind="ExternalInput").ap()
    wv_d = nc.dram_tensor("WvB", [128, 8, 1024], bf16, kind="ExternalInput").ap()
    wo_d = nc.dram_tensor("WoB", [128, 8, 1024], bf16, kind="ExternalInput").ap()
    ct_d = nc.dram_tensor("costan", [128, 2 * S], bf16, kind="ExternalInput").ap()
    rm_d = nc.dram_tensor("rtmask", [128, 384], bf16, kind="ExternalInput").ap()
    out_d = nc.dram_tensor("outT", [D, T], bf16, kind="ExternalOutput").ap()

    KC = D // 128

    with tile.TileContext(nc) as tc:
        with (
            tc.tile_pool(name="consts", bufs=1) as consts,
            tc.tile_pool(name="persist", bufs=1) as persist,
            tc.tile_pool(name="work", bufs=3) as work,
            tc.tile_pool(name="expp", bufs=3) as expp,
            tc.tile_pool(name="ps_a", bufs=4, space="PSUM") as ps_a,
            tc.tile_pool(name="ps_b", bufs=2, space="PSUM") as ps_b,
        ):
            # ---- input loads: few big triggers, critical slabs first ----
            rtmask = consts.tile([128, 384], bf16, name="rtmask")
            nc.sync.dma_start(out=rtmask, in_=rm_d)
            RT = rtmask[:, 0:128]
            mask2 = rtmask[:, 128:384].rearrange("p (h w) -> p h w", w=128)

            wqB = consts.tile([128, 8, 8, 128], bf16, name="wqB")
            wkB = consts.tile([128, 8, 8, 128], bf16, name="wkB")
            xTw = consts.tile([128, 2, 8, S], bf16, name="xTw")
            nc.sync.dma_start(out=wqB[:, 0, :, :], in_=wq_d[0])
            nc.sync.dma_start(out=wkB[:, 0, :, :], in_=wk_d[0])
            nc.sync.dma_start(out=xTw[:, 0, :, :], in_=xT_d[0])
            costan = consts.tile([128, 2 * S], bf16, name="costan")
            nc.sync.dma_start(out=costan, in_=ct_d)
            cosT = costan[:, 0:S]
            tanT = costan[:, S:2 * S]
            nc.sync.dma_start(out=xTw[:, 1, :, :], in_=xT_d[1])
            for m in range(1, 8):
                nc.sync.dma_start(out=wqB[:, m, :, :], in_=wq_d[m])
                nc.sync.dma_start(out=wkB[:, m, :, :], in_=wk_d[m])
            wvB = consts.tile([128, 8, 1024], bf16, name="wvB")
            nc.sync.dma_start(out=wvB, in_=wv_d)
            woB = consts.tile([128, 8, 1024], bf16, name="woB")
            nc.sync.dma_start(out=woB, in_=wo_d)

            # ---- PE warmup on the RT tile (first slab to land) ----
            wps = ps_a.tile([128, S], f32, name="wps", tag="mm", bufs=2)
            for _ in range(36):
                nc.tensor.matmul(wps[:, 0:128], rtmask[:, 0:128], rtmask[:, 0:128],
                                 start=True, stop=True)
            # pre-load the ACT exp table set during the DMA wait
            actw = consts.tile([1, 8], bf16, name="actw")
            nc.scalar.activation(actw, rtmask[0:1, 0:8], Exp, scale=0.125)

            qrot = [persist.tile([128, T], bf16, name=f"qrot{m}") for m in range(KC)]
            krot = [persist.tile([128, T], bf16, name=f"krot{m}") for m in range(KC)]
            vsb = [persist.tile([128, H * 65], bf16, name=f"vsb{t_}") for t_ in range(T // 128)]
            att = [persist.tile([128, T], bf16, name=f"att{m}") for m in range(KC)]

            for t_ in range(T // 128):
                vt = vsb[t_].rearrange("p (h w) -> p h w", w=65)
                nc.gpsimd.memset(vt[:, :, 64:65], 1.0)

            def xcols(nb):
                return xTw[:, nb]  # [128, 8, S]

            # qk projection with the RoPE tan trick (A/B split).
            def emit_qk_A(nb, w_sb, rot, m, tag="mm"):
                xh = xcols(nb)
                pp = ps_a.tile([128, S], f32, name="pp", tag=tag, bufs=2)
                for k in range(KC):
                    nc.tensor.matmul(
                        pp, w_sb[:, m, k, :], xh[:, k, :],
                        start=(k == 0), stop=(k == KC - 1))
                pre2 = work.tile([128, S], bf16, name="pre2", tag="pre2", bufs=2)
                nc.vector.tensor_mul(pre2, pp, tanT)
                return (pp, pre2, rot, m, nb)

            def emit_qk_B(st):
                pp, pre2, rot, m, nb = st
                cols = slice(nb * S, (nb + 1) * S)
                nc.tensor.matmul(pp, RT, pre2, start=False, stop=True,
                                 skip_group_check=True)
                nc.vector.tensor_mul(rot[m][:, cols], pp, cosT)

            def emit_qk_unit(nb, m, tag_q="mm"):
                st_q = emit_qk_A(nb, wqB, qrot, m, tag=tag_q)
                st_k = emit_qk_A(nb, wkB, krot, m)
                emit_qk_B(st_q)
                emit_qk_B(st_k)

            def emit_v_group(tch, nh, on_dve=False):
                vt = vsb[tch].rearrange("p (h w) -> p h w", w=65)
                xh = xcols(tch // 4)
                tl = (tch % 4) * 128
                vp = ps_a.tile([128, S], f32, name="vp", tag="mm", bufs=2)
                for k in range(KC):
                    nc.tensor.matmul(
                        vp, xh[:, k, tl:tl + 128],
                        wvB[:, k, nh * S:(nh + 1) * S],
                        start=(k == 0), stop=(k == KC - 1))
                cp = nc.vector.tensor_copy if on_dve else nc.scalar.copy
                cp(vt[:, nh * 8:(nh + 1) * 8, 0:64],
                   vp.rearrange("p (h w) -> p h w", w=64))

            # attention pair; softmax denominators for BOTH heads share one
            # transpose/recip/transpose-back dance.
            def emit_attn_pair(b, j):
                mh = j
                exs = []
                for i in range(4):
                    lo = i * 128
                    sc = ps_b.tile([128, 2, S], f32, name="sc", tag="ps_b")
                    for hi, p0 in ((0, 0), (1, 64)):
                        nc.tensor.matmul(
                            sc[:, hi, 0:S - lo],
                            krot[mh][p0:p0 + 64, b * S + lo: b * S + lo + 128],
                            qrot[mh][p0:p0 + 64, b * S + lo: (b + 1) * S],
                            start=True, stop=True)
                    ex = expp.tile([128, 2, S], bf16, name="ex", tag=f"ex{i}")
                    nc.scalar.activation(ex[:, :, lo:S], sc[:, :, 0:S - lo], Exp, scale=0.125)
                    nc.vector.tensor_mul(ex[:, :, lo:lo + 128], ex[:, :, lo:lo + 128], mask2)
                    exs.append(ex)

                avs = []
                ss2 = work.tile([1, 2, S], f32, name="ss2", tag="ss2", bufs=2)
                for hi in (0, 1):
                    h = 2 * j + hi
                    av = ps_a.tile([128, S], f32, name="av", tag="av", bufs=2)
                    for i in range(4):
                        lo = i * 128
                        nc.tensor.matmul(
                            av[0:65, lo:S],
                            vsb[b * 4 + i][:, h * 65: h * 65 + 65],
                            exs[i][:, hi, lo:S],
                            start=(i == 0), stop=(i == 3), skip_group_check=True)
                    if hi == 0:
                        nc.scalar.copy(ss2[0:1, 0, :], av[64:65, :])
                    else:
                        nc.vector.tensor_copy(ss2[0:1, 1, :], av[64:65, :])
                    avs.append(av)
                st = work.tile([128, 8], f32, name="st", tag="st", bufs=2)
                nc.gpsimd.dma_start(out=st, in_=ss2)
                rt = work.tile([128, 8], f32, name="rt", tag="rt", bufs=2)
                nc.vector.reciprocal(rt, st)
                rr = work.tile([1, 2, S], f32, name="rr", tag="rr", bufs=2)
                nc.gpsimd.dma_start(out=rr, in_=rt)
                bcols = slice(b * S, (b + 1) * S)
                for hi in (0, 1):
                    p0 = hi * 64
                    rb = work.tile([64, S], f32, name="rb", tag="rb", bufs=2)
                    nc.gpsimd.partition_broadcast(rb, rr[0:1, hi, :])
                    nc.vector.tensor_mul(att[mh][p0:p0 + 64, bcols], avs[hi][0:64, :], rb)

            def emit_wo_group(b, m):
                bcols = slice(b * S, (b + 1) * S)
                fin = ps_a.tile([128, S], f32, name="fin", tag="mm", bufs=2)
                for k in range(KC):
                    nc.tensor.matmul(
                        fin, woB[:, k, m * 128:(m + 1) * 128], att[k][:, bcols],
                        start=(k == 0), stop=(k == KC - 1))
                ob = work.tile([128, S], bf16, name="ob", tag="ob", bufs=2)
                nc.scalar.copy(ob, fin)
                nc.sync.dma_start(out=out_d[m * 128:(m + 1) * 128, bcols], in_=ob)

            # ---- global schedule ----
            for m in range(KC):
                emit_qk_unit(0, m, tag_q="av")
            for tch in range(4):
                emit_v_group(tch, 0)
            for j in range(4):  # pairs (0,0..3)
                emit_attn_pair(0, j)
                emit_v_group(j, 1)
                emit_qk_unit(1, j)
            for j in range(4, 8):  # pairs (0,4..7)
                emit_attn_pair(0, j)
                emit_v_group(j, 0)
                emit_qk_unit(1, j)
            for j in range(4):  # pairs (1,0..3)
                emit_attn_pair(1, j)
                emit_v_group(4 + j, 1, on_dve=True)
                emit_wo_group(0, j)
            for j in range(4, 8):  # pairs (1,4..7)
                emit_wo_group(0, j)
                emit_attn_pair(1, j)
            for m in range(KC):
                emit_wo_group(1, m)

    nc.compile()
    return nc


def _get_nc():
    if "nc" not in _CACHE:
        _CACHE["nc"] = _build_bass()
    return _CACHE["nc"]


def make_in_maps(x, Wq, Wk, Wv, Wo):
    costan, rtmask = _host_consts()
    shared = {
        "WqB": _block_weight(Wq),
        "WkB": _block_weight(Wk),
        "WvB": _kmajor(Wv),
        "WoB": _kmajor(Wo),
        "costan": costan,
        "rtmask": rtmask,
    }
    in_maps = []
    for c in range(NCORES):
        xc = x[c * BPC:(c + 1) * BPC]
        xT = np.ascontiguousarray(xc.transpose(2, 0, 1).reshape(D, T)).astype(BF16)
        # [nb, p, k, t] layout: per batch-half, contiguous [128, 8, S] slab
        xkpt = xT.reshape(8, 128, 2, S)                   # [k, p, nb, t]
        xTw = np.ascontiguousarray(xkpt.transpose(2, 1, 0, 3))  # [nb, p, k, t]
        in_maps.append({"xTw": xTw, **shared})
    return in_maps


def assemble(results):
    out = np.empty((B, S, D), np.float32)
    for c in range(NCORES):
        oT = np.asarray(results[c]["outT"]).astype(np.float32)
        out[c * BPC:(c + 1) * BPC] = oT.reshape(D, BPC, S).transpose(1, 2, 0)
    return out


def run(x, Wq, Wk, Wv, Wo, trace=False, **run_kwargs):
    from concourse.bass_utils import run_bass_kernel_spmd
    nc = _get_nc()
    in_maps = make_in_maps(x, Wq, Wk, Wv, Wo)
    res = run_bass_kernel_spmd(
        nc, in_maps, core_ids=list(range(NCORES)), trace=trace, **run_kwargs)
    return assemble(res.results), res


def kernel(x, Wq, Wk, Wv, Wo):
    out, _ = run(np.asarray(x), np.asarray(Wq), np.asarray(Wk),
                 np.asarray(Wv), np.asarray(Wo))
    return out


# revision 24
# speedup vs baseline: 1.1877x; 1.1877x over previous
"""Multi-head attention (RoPE + causal softmax) Trainium2 Bass kernel.

Problem: nn_MultiHeadAttention (B=16, S=512, D=1024, H=16, Hd=64).
Sharding: data-parallel over batch — 2 batches per core on 8 NeuronCores.

Feature-major device layout ([feature, token] tiles) so the contraction
always sits on SBUF partitions.  Per-core pipeline: q/k projections with
RoPE, v projection (ones-column augmented so attn@v also yields softmax
denominators), per-head-pair causal attention with row-group-packed score
matmuls, Wo projection.  Emission interleaves attention pairs with dense
projection/wo matmul groups so the PE stays warm (HAM K=8/8).

RoPE uses the tan identity: because the RoPE tables repeat with period 32,
R @ (q * tan) * cos == rotate_half(q) * sin, so the rotation matmul
accumulates directly into the projection PSUM (start=False continuation)
and RoPE costs just 2 DVE ops per group.  The rotation matmul is emitted
one group late (A/B split) so its pre2 dependency hides behind 8 dense
matmuls.  The causal diagonal mask is a single [128, 2, 128] bf16 DVE
multiply covering both heads of a pair.

v10 additions over the 227us baseline (trace-driven):
- Input DMA staging: each dma_start trigger costs ~0.5us of serial Sync
  time and one ring's packets spread across all 16 DMA engines, so inputs
  are staged host-side as contiguous slabs and loaded with ~22 big
  triggers ordered critical-first (wq/wk m=0 slices, batch-0 half of x,
  cos/tan, then the rest).
- PE warmup matmuls on the RT tile bridge the framework preamble to first
  data arrival so HAM is at K=8/8 when real matmuls start (baseline spent
  its first 27.8us at 1.2 GHz); the ACT exp table pre-loads in the same
  window.
- cos/tan tables halved to [128, S] (they repeat across the 2 batches).
- Output stored bf16 (tolerance 2e-2 >> bf16 rounding; halves out-DMA).
"""

import numpy as np
import ml_dtypes

BF16 = ml_dtypes.bfloat16

B, S, D = 16, 512, 1024
H, HD = 16, 64
NCORES = 8
BPC = B // NCORES
T = BPC * S

_CACHE = {}


def _rope_tables():
    inv_freq = 1.0 / (10000.0 ** (np.arange(0, HD, 2, dtype=np.float64) / HD))
    t = np.arange(S, dtype=np.float64)
    freqs = np.outer(t, inv_freq)
    emb = np.concatenate([freqs, freqs], -1)
    return np.cos(emb), np.sin(emb)


def _host_consts():
    cos, sin = _rope_tables()
    tan = sin / cos
    cosT = np.tile(cos.T, (2, 1))
    tanT = np.tile(tan.T, (2, 1))
    costan = np.ascontiguousarray(
        np.concatenate([cosT, tanT], axis=1)).astype(BF16)  # [128, 2S]
    R64 = np.zeros((64, 64), np.float32)
    R64[np.arange(32), np.arange(32) + 32] = -1.0
    R64[np.arange(32) + 32, np.arange(32)] = 1.0
    R128 = np.zeros((128, 128), np.float32)
    R128[:64, :64] = R64
    R128[64:, 64:] = R64
    RT = R128.T.astype(BF16)
    mask01 = (np.arange(128)[None, :] >= np.arange(128)[:, None]).astype(BF16)
    mask2 = np.concatenate([mask01, mask01], axis=1)     # [128, 256]
    rtmask = np.ascontiguousarray(
        np.concatenate([RT, mask2], axis=1))              # [128, 384]
    return costan, rtmask


def _block_weight(w):
    """WqT [D, D] -> [8, 128, 1024] m-major contiguous slabs:
    out[m, p, k*128 + c] = WqT[k*128+p, m*128+c]."""
    wt = np.ascontiguousarray(w.T).astype(BF16)
    b = wt.reshape(8, 128, 8, 128)                        # [k, p, m, c]
    return np.ascontiguousarray(
        b.transpose(2, 1, 0, 3).reshape(8, 128, 1024))    # [m][p, k, c]


def _kmajor(w):
    """WvT [D, D] -> [128, 8, 1024]: out[p, k, c] = WvT[k*128+p, c]."""
    wt = np.ascontiguousarray(w.T).astype(BF16)
    return np.ascontiguousarray(
        wt.reshape(8, 128, 1024).transpose(1, 0, 2))


def _build_bass(dump_debug=False):
    import concourse.bacc as bacc
    import concourse.tile as tile
    import concourse.mybir as mybir

    dt = mybir.dt
    f32, bf16 = dt.float32, dt.bfloat16
    Exp = mybir.ActivationFunctionType.Exp

    nc = bacc.Bacc("TRN2", target_bir_lowering=False, debug=False, enable_asserts=False)

    xT_d = nc.dram_tensor("xTw", [2, 128, 8, S], bf16, kind="ExternalInput").ap()
    wq_d = nc.dram_tensor("WqB", [8, 128, 1024], bf16, kind="ExternalInput").ap()
    wk_d = nc.dram_tensor("WkB", [8, 128, 1024], bf16, kind="ExternalInput").ap()
    wv_d = nc.dram_tensor("WvB", [128, 8, 1024], bf16, kind="ExternalInput").ap()
    wo_d = nc.dram_tensor("WoB", [128, 8, 1024], bf16, kind="ExternalInput").ap()
    ct_d = nc.dram_tensor("costan", [128, 2 * S], bf16, kind="ExternalInput").ap()
    rm_d = nc.dram_tensor("rtmask", [128, 384], bf16, kind="ExternalInput").ap()
    out_d = nc.dram_tensor("outT", [D, T], bf16, kind="ExternalOutput").ap()

    KC = D // 128

    with tile.TileContext(nc) as tc:
        with (
            tc.tile_pool(name="consts", bufs=1) as consts,
            tc.tile_pool(name="persist", bufs=1) as persist,
            tc.tile_pool(name="work", bufs=3) as work,
            tc.tile_pool(name="expp", bufs=3) as expp,
            tc.tile_pool(name="ps_a", bufs=4, space="PSUM") as ps_a,
            tc.tile_pool(name="ps_b", bufs=2, space="PSUM") as ps_b,
        ):
            # ---- input loads: few big triggers, critical slabs first ----
            rtmask = consts.tile([128, 384], bf16, name="rtmask")
            nc.sync.dma_start(out=rtmask, in_=rm_d)
            RT = rtmask[:, 0:128]
            mask2 = rtmask[:, 128:384].rearrange("p (h w) -> p h w", w=128)

            wqB = consts.tile([128, 8, 8, 128], bf16, name="wqB")
            wkB = consts.tile([128, 8, 8, 128], bf16, name="wkB")
            xTw = consts.tile([128, 2, 8, S], bf16, name="xTw")
            nc.sync.dma_start(out=wqB[:, 0, :, :], in_=wq_d[0])
            nc.sync.dma_start(out=wkB[:, 0, :, :], in_=wk_d[0])
            nc.sync.dma_start(out=xTw[:, 0, :, :], in_=xT_d[0])
            costan = consts.tile([128, 2 * S], bf16, name="costan")
            nc.sync.dma_start(out=costan, in_=ct_d)
            cosT = costan[:, 0:S]
            tanT = costan[:, S:2 * S]
            nc.sync.dma_start(out=xTw[:, 1, :, :], in_=xT_d[1])
            for m in range(1, 8):
                nc.sync.dma_start(out=wqB[:, m, :, :], in_=wq_d[m])
                nc.sync.dma_start(out=wkB[:, m, :, :], in_=wk_d[m])
            wvB = consts.tile([128, 8, 1024], bf16, name="wvB")
            nc.sync.dma_start(out=wvB, in_=wv_d)
            woB = consts.tile([128, 8, 1024], bf16, name="woB")
            nc.sync.dma_start(out=woB, in_=wo_d)

            # ---- PE warmup on the RT tile (first slab to land) ----
            wps = ps_a.tile([128, S], f32, name="wps", tag="ps_a")
            for _ in range(30):
                nc.tensor.matmul(wps[:, 0:128], rtmask[:, 0:128], rtmask[:, 0:128],
                                 start=True, stop=True)
            # pre-load the ACT exp table set during the DMA wait
            actw = consts.tile([1, 8], bf16, name="actw")
            nc.scalar.activation(actw, rtmask[0:1, 0:8], Exp, scale=0.125)

            qrot = [persist.tile([128, T], bf16, name=f"qrot{m}") for m in range(KC)]
            krot = [persist.tile([128, T], bf16, name=f"krot{m}") for m in range(KC)]
            vsb = [persist.tile([128, H * 65], bf16, name=f"vsb{t_}") for t_ in range(T // 128)]
            att = [persist.tile([128, T], bf16, name=f"att{m}") for m in range(KC)]

            for t_ in range(T // 128):
                vt = vsb[t_].rearrange("p (h w) -> p h w", w=65)
                nc.gpsimd.memset(vt[:, :, 64:65], 1.0)

            # qk projection with the RoPE tan trick (A/B split).
            def emit_qk_A(nb, w_sb, rot, m):
                xh = xTw[:, nb]
                pp = ps_a.tile([128, S], f32, name="pp", tag="ps_a")
                for k in range(KC):
                    nc.tensor.matmul(
                        pp, w_sb[:, m, k, :], xh[:, k, :],
                        start=(k == 0), stop=(k == KC - 1))
                pre2 = work.tile([128, S], bf16, name="pre2", tag="pre2", bufs=2)
                nc.vector.tensor_mul(pre2, pp, tanT)
                return (pp, pre2, rot, m, nb)

            def emit_qk_B(st):
                pp, pre2, rot, m, nb = st
                cols = slice(nb * S, (nb + 1) * S)
                nc.tensor.matmul(pp, RT, pre2, start=False, stop=True,
                                 skip_group_check=True)
                nc.vector.tensor_mul(rot[m][:, cols], pp, cosT)

            def emit_qk_unit(nb, m):
                st_q = emit_qk_A(nb, wqB, qrot, m)
                st_k = emit_qk_A(nb, wkB, krot, m)
                emit_qk_B(st_q)
                emit_qk_B(st_k)

            def emit_v_group(tch, nh):
                vt = vsb[tch].rearrange("p (h w) -> p h w", w=65)
                xh = xTw[:, tch // 4]
                tl = (tch % 4) * 128
                vp = ps_a.tile([128, S], f32, name="vp", tag="ps_a")
                for k in range(KC):
                    nc.tensor.matmul(
                        vp, xh[:, k, tl:tl + 128],
                        wvB[:, k, nh * S:(nh + 1) * S],
                        start=(k == 0), stop=(k == KC - 1))
                nc.scalar.copy(
                    vt[:, nh * 8:(nh + 1) * 8, 0:64],
                    vp.rearrange("p (h w) -> p h w", w=64))

            def emit_attn_head(b, h, exs):
                bcols = slice(b * S, (b + 1) * S)
                mh, p0 = h // 2, (h % 2) * 64
                hi = h % 2
                av = ps_a.tile([128, S], f32, name="av", tag="ps_a")
                for i in range(4):
                    lo = i * 128
                    nc.tensor.matmul(
                        av[0:65, lo:S],
                        vsb[b * 4 + i][:, h * 65: h * 65 + 65],
                        exs[i][:, hi, lo:S],
                        start=(i == 0), stop=(i == 3), skip_group_check=True)
                ss = work.tile([1, S], f32, name="ss", tag="ss")
                nc.vector.tensor_copy(ss, av[64:65, :])
                st = work.tile([128, 4], f32, name="st", tag="st")
                nc.gpsimd.dma_start(out=st, in_=ss)
                rt = work.tile([128, 4], f32, name="rt", tag="rt")
                nc.vector.reciprocal(rt, st)
                rr = work.tile([1, S], f32, name="rr", tag="rr")
                nc.gpsimd.dma_start(out=rr, in_=rt)
                rb = work.tile([64, S], f32, name="rb", tag="rb", bufs=2)
                nc.gpsimd.partition_broadcast(rb, rr)
                nc.vector.tensor_mul(att[mh][p0:p0 + 64, bcols], av[0:64, :], rb)

            def emit_attn_pair(b, j):
                mh = j
                exs = []
                for i in range(4):
                    lo = i * 128
                    sc = ps_b.tile([128, 2, S], f32, name="sc", tag="ps_b")
                    for hi, p0 in ((0, 0), (1, 64)):
                        nc.tensor.matmul(
                            sc[:, hi, 0:S - lo],
                            krot[mh][p0:p0 + 64, b * S + lo: b * S + lo + 128],
                            qrot[mh][p0:p0 + 64, b * S + lo: (b + 1) * S],
                            start=True, stop=True)
                    ex = expp.tile([128, 2, S], bf16, name="ex", tag=f"ex{i}")
                    nc.scalar.activation(ex[:, :, lo:S], sc[:, :, 0:S - lo], Exp, scale=0.125)
                    nc.vector.tensor_mul(ex[:, :, lo:lo + 128], ex[:, :, lo:lo + 128], mask2)
                    exs.append(ex)
                emit_attn_head(b, 2 * j, exs)
                emit_attn_head(b, 2 * j + 1, exs)

            def emit_wo_group(b, m):
                bcols = slice(b * S, (b + 1) * S)
                fin = ps_a.tile([128, S], f32, name="fin", tag="ps_a")
                for k in range(KC):
                    nc.tensor.matmul(
                        fin, woB[:, k, m * 128:(m + 1) * 128], att[k][:, bcols],
                        start=(k == 0), stop=(k == KC - 1))
                ob = work.tile([128, S], bf16, name="ob", tag="ob", bufs=2)
                nc.vector.tensor_copy(ob, fin)
                nc.sync.dma_start(out=out_d[m * 128:(m + 1) * 128, bcols], in_=ob)

            # ---- global schedule (baseline) ----
            for m in range(KC):
                emit_qk_unit(0, m)
            for tch in range(4):
                for nh in range(2):
                    emit_v_group(tch, nh)
            v1 = [(tch, nh) for tch in range(4, 8) for nh in range(2)]
            for j in range(H // 2):
                emit_attn_pair(0, j)
                emit_qk_unit(1, j)
                emit_v_group(*v1[j])
            for j in range(H // 2):
                emit_attn_pair(1, j)
                emit_wo_group(0, j)
            for m in range(KC):
                emit_wo_group(1, m)

    nc.compile()
    return nc


def _get_nc():
    if "nc" not in _CACHE:
        _CACHE["nc"] = _build_bass()
    return _CACHE["nc"]


def make_in_maps(x, Wq, Wk, Wv, Wo):
    costan, rtmask = _host_consts()
    shared = {
        "WqB": _block_weight(Wq),
        "WkB": _block_weight(Wk),
        "WvB": _kmajor(Wv),
        "WoB": _kmajor(Wo),
        "costan": costan,
        "rtmask": rtmask,
    }
    in_maps = []
    for c in range(NCORES):
        xc = x[c * BPC:(c + 1) * BPC]
        xT = np.ascontiguousarray(xc.transpose(2, 0, 1).reshape(D, T)).astype(BF16)
        xkpt = xT.reshape(8, 128, 2, S)                   # [k, p, nb, t]
        xTw = np.ascontiguousarray(xkpt.transpose(2, 1, 0, 3))  # [nb, p, k, t]
        in_maps.append({"xTw": xTw, **shared})
    return in_maps


def assemble(results):
    out = np.empty((B, S, D), np.float32)
    for c in range(NCORES):
        oT = np.asarray(results[c]["outT"]).astype(np.float32)
        out[c * BPC:(c + 1) * BPC] = oT.reshape(D, BPC, S).transpose(1, 2, 0)
    return out


def run(x, Wq, Wk, Wv, Wo, trace=False, **run_kwargs):
    from concourse.bass_utils import run_bass_kernel_spmd
    nc = _get_nc()
    in_maps = make_in_maps(x, Wq, Wk, Wv, Wo)
    res = run_bass_kernel_spmd(
        nc, in_maps, core_ids=list(range(NCORES)), trace=trace, **run_kwargs)
    return assemble(res.results), res


def kernel(x, Wq, Wk, Wv, Wo):
    out, _ = run(np.asarray(x), np.asarray(Wq), np.asarray(Wk),
                 np.asarray(Wv), np.asarray(Wo))
    return out
